# revision 10
# baseline (speedup 1.0000x reference)
"""Locally-connected layer (unshared 3x3 conv, torch-unfold semantics) on 8 trn2 cores.

out[b,o,y,x] = sum_{c,i,j} weight[o, c*9+i*3+j, y*32+x] * xpad[b, c, y+i, x+j]

Sharding: spatial over L — core r owns image rows [4r, 4r+4) (128 pixels).

v3 design (bf16, weights-stationary, N=128 moving, single slab tile):
  * Everything bf16 on the wire (tolerance 2e-2; bf16 error ~1%). PSUM fp32.
  * SBUF slab T1 [128, B*204] = [slab | slab shifted +1 col] (host-built,
    one contiguous HBM DMA).  All im2col is pure access-pattern offsets.
  * The 576-long contraction is reordered into chunks whose stationary is a
    [K, 128] host-packed weight block covering BOTH pixels of a pair
    (cols m = 64*e + o: pixel parity e, channel o).  Moving operand is
    x [K, N=128=(b, pix)] read as t13[:, :, off:off+2].  PSUM [128, (b,pix)]:
    only the e==pix halves are read out, the rest is discarded.
      q0..q2: K=128  rows [c x (i=q,j=0) | c x (i=q,j=1)]  rhs t13[0:128] @ off
      s0..s2: K=64   rows  c x (s,2)  (ws blob, rows 0:64) rhs t13[0:64]
    All matmuls sit at row base 0 (mixed-base accumulation groups crash TRN2).
  * 6 matmuls / 6 ldweights per pixel pair (384 per core), all with 128-col
    stationaries (FWL-eligible) and N=128 moving.  PSUM readout alternates
    vector/scalar by pair so the two engines touch different PSUM banks.
  * Output bf16 in [psum-partition, pair, b] DRAM layout (contiguous DMA);
    host transposes to (B, O, H, W) fp32.
"""

import numpy as np
import ml_dtypes

BF16 = ml_dtypes.bfloat16

B, C, O, H, W, KS = 64, 64, 64, 32, 32, 3
L = H * W
NCORES = 8
RPC = H // NCORES            # image rows per core = 4
LC = RPC * W                 # pixels per core = 128
NP = LC // 2                 # pixel pairs per core = 64
HALO = RPC + 2               # 6 slab rows
WP = W + 2                   # padded width 34
BST = HALO * WP              # per-b free stride in the slab = 204
PG = 8                       # pairs per weight DMA group
NG = NP // PG                # weight groups = 8

_CACHE = {}


def _build_nc():
    import concourse.bass as bass
    import concourse.bacc as bacc
    import concourse.tile as tile
    from concourse import mybir

    f32 = mybir.dt.float32
    bf16 = mybir.dt.bfloat16
    nc = bacc.Bacc(
        "TRN2", target_bir_lowering=False, debug=False, num_devices=NCORES
    )
    x_d = nc.dram_tensor("x", [64, B * BST], bf16, kind="ExternalInput")
    wq_d = nc.dram_tensor("wq", [NG, 128, PG, 3, 128], bf16, kind="ExternalInput")
    ws_d = nc.dram_tensor("ws", [NG, 64, PG, 3, 128], bf16, kind="ExternalInput")
    o_d = nc.dram_tensor("out", [128, NP, B], bf16, kind="ExternalOutput")

    with tile.TileContext(nc) as tc:
        with (
            tc.tile_pool(name="x1", bufs=1) as x1pool,
            tc.tile_pool(name="wq", bufs=4) as wpool,
            tc.tile_pool(name="ws", bufs=4) as spool,
            tc.tile_pool(name="orow", bufs=2) as opool,
            tc.tile_pool(name="ps", bufs=8, space=bass.MemorySpace.PSUM) as pspool,
        ):
            t1 = x1pool.tile([128, B * BST], bf16)
            t13 = t1[:].rearrange("p (b f) -> p b f", f=BST)
            x_v = x_d[:].rearrange("p (b f) -> p b f", f=BST)
            # HBM sends only the lower slab (4 b-sliced DMAs flood parallel
            # queues); the +1-col shifted upper half is an on-chip copy.
            BQ = B // 4
            for i in range(4):
                bs = slice(i * BQ, (i + 1) * BQ)
                nc.sync.dma_start(t13[0:64, bs], x_v[:, bs])
            for i in range(4):
                bs = slice(i * BQ, (i + 1) * BQ)
                nc.sync.dma_start(
                    t13[64:128, bs, 0 : BST - 1], t13[0:64, bs, 1:BST]
                )

            for g in range(NG):
                wt = wpool.tile([128, PG, 3, 128], bf16)
                st = spool.tile([64, PG, 3, 128], bf16)
                nc.sync.dma_start(wt[:], wq_d[g])
                nc.sync.dma_start(st[:], ws_d[g])
                orow = opool.tile([128, PG, B], bf16)
                for tt in range(PG):
                    t = g * PG + tt
                    y, x0 = (2 * t) // W, (2 * t) % W
                    ps = pspool.tile([128, B, 2], f32)
                    for q in range(3):
                        off = (y + q) * WP + x0
                        nc.tensor.matmul(
                            ps[:], wt[:, tt, q, :], t13[:, :, off : off + 2],
                            start=(q == 0), stop=False,
                        )
                    for s in range(3):
                        offs = (y + s) * WP + x0 + 2
                        nc.tensor.matmul(
                            ps[:], st[:, tt, s, :], t13[0:64, :, offs : offs + 2],
                            start=False, stop=(s == 2),
                        )
                    eng = nc.vector if tt % 2 == 0 else nc.scalar
                    if tt % 2 == 0:
                        nc.vector.tensor_copy(orow[0:64, tt, :], ps[0:64, :, 0])
                        nc.vector.tensor_copy(orow[64:128, tt, :], ps[64:128, :, 1])
                    else:
                        nc.scalar.copy(orow[0:64, tt, :], ps[0:64, :, 0])
                        nc.scalar.copy(orow[64:128, tt, :], ps[64:128, :, 1])
                nc.sync.dma_start(o_d[:, g * PG : (g + 1) * PG, :], orow[:])
    nc.compile()
    return nc


def _get_nc():
    if "nc" not in _CACHE:
        _CACHE["nc"] = _build_nc()
    return _CACHE["nc"]


def _pack_x(x):
    """Per core: [64, B*BST] bf16 lower slab (the +1-col shifted upper half
    of the on-chip tile is built by an SBUF->SBUF DMA inside the kernel)."""
    xpad = np.pad(x, ((0, 0), (0, 0), (1, 1), (1, 1)))
    xpad = np.ascontiguousarray(xpad.transpose(1, 0, 2, 3))  # [C, B, 34, 34]
    outs = []
    for r in range(NCORES):
        slab = xpad[:, :, RPC * r : RPC * r + HALO, :].reshape(C, B, BST)
        outs.append(np.ascontiguousarray(slab.astype(BF16).reshape(64, B * BST)))
    return outs


def _pack_w(weight):
    """Chunked-contraction weight blobs, already in SBUF layout.

    wq: [core, NG, p=(j, c), tt, q, m=(e, o)]   (pair chunks, shifts (q, j))
    ws: [core, NG, c, tt, s, m=(e, o)]          (singles, shifts (s, 2))
    """
    w5 = weight.reshape(O, C, KS, KS, L)
    low = np.stack([w5[:, :, 0, 0], w5[:, :, 1, 0], w5[:, :, 2, 0]], axis=0)
    up = np.stack([w5[:, :, 0, 1], w5[:, :, 1, 1], w5[:, :, 2, 1]], axis=0)
    wq = np.stack([low, up], axis=1)          # [q, j, O, C, L]
    wq = wq.reshape(3, 2, O, C, NCORES, NG, PG, 2)
    # -> [core, g, j, c, tt, q, e, o]
    wq = wq.transpose(4, 5, 1, 3, 6, 0, 7, 2)
    wq = np.ascontiguousarray(wq, dtype=BF16).reshape(NCORES, NG, 128, PG, 3, 128)

    ws = np.stack([w5[:, :, 0, 2], w5[:, :, 1, 2], w5[:, :, 2, 2]], axis=0)
    ws = ws.reshape(3, O, C, NCORES, NG, PG, 2)
    ws = ws.transpose(3, 4, 2, 5, 0, 6, 1)    # [core, g, c, tt, s, e, o]
    ws = np.ascontiguousarray(ws, dtype=BF16).reshape(NCORES, NG, 64, PG, 3, 128)
    return wq, ws


def kernel(x, weight, bias, _trace=False, _trace_kwargs=None):
    from concourse.bass_utils import run_bass_kernel_spmd

    x = np.asarray(x, dtype=np.float32)
    weight = np.asarray(weight, dtype=np.float32)
    bias = np.asarray(bias, dtype=np.float32)

    nc = _get_nc()
    xs = _pack_x(x)
    wq, ws = _pack_w(weight)
    in_maps = [
        {"x": xs[r], "wq": wq[r], "ws": ws[r]} for r in range(NCORES)
    ]
    res = run_bass_kernel_spmd(
        nc, in_maps, list(range(NCORES)),
        trace=_trace, **(_trace_kwargs or {}),
    )
    # out[r]: [p=(e,o), t, b] bf16 -> [b, o, l=128r+2t+e]
    parts = []
    for r in range(NCORES):
        arr = res.results[r]["out"].astype(np.float32)
        arr = arr.reshape(2, O, NP, B).transpose(3, 1, 2, 0)  # [b, o, t, e]
        parts.append(arr.reshape(B, O, LC))
    out = np.concatenate(parts, axis=2).reshape(B, O, H, W)
    if np.any(bias):
        out = out + bias.reshape(1, O, H, W)
    if _trace:
        _CACHE["last_result"] = res
    return np.ascontiguousarray(out.astype(np.float32))
